# revision 1
# baseline (speedup 1.0000x reference)
"""Trainium2 Bass kernel for nn_MicResponseAugment: HP(125Hz)+LP(6kHz) biquad
cascade over waveform [128, 160000] f32.

Algorithm: the biquad cascade is an LTI filter whose impulse response decays
like r^n with r = 0.9659 (HP pole radius).  Truncating at >= 385 taps gives
relative L2 error ~1e-7 — far below the fp32 noise floor of the reference
itself — so the problem becomes a causal FIR computed on the PE as
block-Toeplitz matmuls:

    y[m*128 + i] = sum_{s=0..3} sum_u  C_s[u, i] * x[(m-s)*128 + u]
    C_s[u, i] = h[s*128 + i - u]   (h = cascade impulse response, h[<0] = 0)

Sharding: data-parallel over channels — core c handles channels
[16c, 16c+16).  Per channel the time axis is 1250 blocks of 128 samples,
processed as 10 transpose-tiles of 125 blocks (1250 = 10*125, remainder-free).
PE transposes move each block onto the partition dim; the FIR runs as 4
accumulating matmuls per output chunk — C_0/C_1 (which hold all taps with
|h| > 8e-4) in exact fp32 at 4 cyc/row, C_2/C_3 in float32r at 1 cyc/row
(f32r noises at ~2.4e-4 of each term's own output scale, harmless only for
the small tail taps); PE transposes then restore the natural layout.  DMA
is batched per channel (HWDGE cost is ~625ns per dma_start, so few big
transfers win), and PSUM transpose staging keeps a single reader so banks
recycle fast.  Measured vs the reference: absmax 2.95e-05 = 1.1x the
reference's own fp32-vs-fp64 envelope; TimelineSim cost model: ~131.6 us.
"""

import numpy as np
from contextlib import ExitStack

import concourse.bacc as bacc
import concourse.bass as bass
import concourse.tile as tile
from concourse import mybir
from concourse.bass_utils import run_bass_kernel_spmd

# ---------------------------------------------------------------- constants
SR = 16000
HP_FREQ = 125.0
LP_FREQ = 6000.0
Q_FACT = 0.7071067811865476

N_CORES = 8
C_TOTAL = 128
T_TOTAL = 160000
CH = C_TOTAL // N_CORES          # 16 channels per core
U = 128                          # FIR block length
QB = T_TOTAL // U                # 1250 blocks per channel
TB = 125                         # blocks per transpose tile
NT = QB // TB                    # 10 transpose tiles per channel
PAD = 4                          # zero-history columns per channel
NTAP_BLK = 4                     # tap-block matmuls; sample i covers taps [0, 385+i)
# FIR output chunks in block units (f32r matmuls require even free size)
CHUNKS = [(0, 418), (418, 416), (834, 416)]
# transpose groups: tiles per psum batch
TGROUPS = [(0, 4), (4, 4), (8, 2)]

F32 = mybir.dt.float32


def _impulse_response(n: int) -> np.ndarray:
    """Cascade impulse response, float64."""
    def coeffs(freq, highpass):
        w0 = 2.0 * np.pi * freq / SR
        cw, sw = np.cos(w0), np.sin(w0)
        al = sw / (2.0 * Q_FACT)
        if highpass:
            b = np.array([(1 + cw) / 2, -(1 + cw), (1 + cw) / 2])
        else:
            b = np.array([(1 - cw) / 2, (1 - cw), (1 - cw) / 2])
        a = np.array([1 + al, -2 * cw, 1 - al])
        # match the reference: coefficients are rounded to fp32 first
        b = (b / a[0]).astype(np.float32).astype(np.float64)
        a = (a / a[0]).astype(np.float32).astype(np.float64)
        return b, a

    def filt(x, b, a):
        y = np.zeros_like(x)
        for i in range(len(x)):
            acc = b[0] * x[i]
            if i >= 1:
                acc += b[1] * x[i - 1] - a[1] * y[i - 1]
            if i >= 2:
                acc += b[2] * x[i - 2] - a[2] * y[i - 2]
            y[i] = acc
        return y

    bh, ah = coeffs(HP_FREQ, True)
    bl, al = coeffs(LP_FREQ, False)
    x = np.zeros(n)
    x[0] = 1.0
    return filt(filt(x, bh, ah), bl, al)


def _toeplitz_weights() -> np.ndarray:
    """cmat[u, s*128 + i] = h[s*128 + i - u], shape [128, 384] f32."""
    h = _impulse_response(NTAP_BLK * U)
    cmat = np.zeros((U, NTAP_BLK * U), dtype=np.float64)
    u = np.arange(U)[:, None]
    i = np.arange(U)[None, :]
    for s in range(NTAP_BLK):
        j = s * U + i - u
        blk = np.where((j >= 0) & (j < NTAP_BLK * U),
                       h[np.clip(j, 0, NTAP_BLK * U - 1)], 0.0)
        cmat[:, s * U:(s + 1) * U] = blk
    return cmat.astype(np.float32)


# ---------------------------------------------------------------- program
F32R = mybir.dt.float32r


def _build_program():
    nc = bacc.Bacc("TRN2", target_bir_lowering=False, debug=False)
    x = nc.dram_tensor("x", [CH, T_TOTAL], F32, kind="ExternalInput")
    cmat_d = nc.dram_tensor("cmat", [U, NTAP_BLK * U], F32, kind="ExternalInput")
    ident_d = nc.dram_tensor("ident", [U, U], F32, kind="ExternalInput")
    y = nc.dram_tensor("y", [CH, T_TOTAL], F32, kind="ExternalOutput")

    # [ch, p(block-in-tile), t(tile), u] view; blocks of tile t are
    # q = t*125 + p, sample = q*128 + u
    x_r = x.ap().rearrange("c (t p u) -> c p t u", t=NT, p=TB, u=U)
    y_r = y.ap().rearrange("c (t p u) -> c p t u", t=NT, p=TB, u=U)

    with tile.TileContext(nc) as tc:
        with ExitStack() as ctx:
            const_p = ctx.enter_context(tc.tile_pool(name="const", bufs=1))
            xa_p = ctx.enter_context(tc.tile_pool(name="xa", bufs=4))
            xt_p = ctx.enter_context(tc.tile_pool(name="xt", bufs=3))
            xl_p = ctx.enter_context(tc.tile_pool(name="xl", bufs=3))
            ytf_p = ctx.enter_context(tc.tile_pool(name="ytf", bufs=3))
            yn_p = ctx.enter_context(tc.tile_pool(name="yn", bufs=4))
            ptg_ps = ctx.enter_context(tc.tile_pool(name="ptg", bufs=3, space="PSUM"))
            pog_ps = ctx.enter_context(tc.tile_pool(name="pog", bufs=2, space="PSUM"))
            fir_ps = ctx.enter_context(tc.tile_pool(name="fir", bufs=2, space="PSUM"))

            # ident first (needed by the first transposes); cmat is DMA'd
            # after channel 0's input pieces so it doesn't hold the HWDGE
            # (625ns serial grant per dma_start) ahead of the critical path
            ident = const_p.tile([U, U], F32)
            nc.sync.dma_start(ident[:], ident_d.ap()[:])
            cmat_raw = const_p.tile([U, NTAP_BLK * U], F32)
            cmat_hi = const_p.tile([U, NTAP_BLK * U], F32R)

            for ch in range(CH):
                # ---- stage A: one DMA + 10 transposes into X_T [128, PAD+QB]
                tgroups = [(0, 2), (2, 4), (6, 4)] if ch == 0 else TGROUPS
                xa = xa_p.tile([TB, NT * U], F32)
                for g0, gn in tgroups:
                    nc.sync.dma_start(
                        xa[:, g0 * U:(g0 + gn) * U].rearrange(
                            "p (t u) -> p t u", u=U),
                        x_r[ch, :, g0:g0 + gn])
                if ch == 0:
                    nc.sync.dma_start(cmat_raw[:], cmat_d.ap()[:])
                    nc.vector.tensor_copy(cmat_hi[:], cmat_raw[:])
                xt_f32 = xt_p.tile([U, PAD + QB], F32)
                nc.vector.memset(xt_f32[:, 0:PAD], 0)
                xt_hi = xl_p.tile([U, PAD + QB], F32R)
                nc.vector.memset(xt_hi[:, 0:PAD].bitcast(F32), 0)
                ytf = ytf_p.tile([U, QB], F32)

                def in_group(g0, gn):
                    ptg = ptg_ps.tile([U, 512], F32, tag="ptg")
                    for k in range(gn):
                        nc.tensor.transpose(
                            ptg[:, 128 * k:128 * k + TB],
                            xa[:, (g0 + k) * U:(g0 + k + 1) * U],
                            ident[:TB, :TB])
                    # batched PSUM->SBUF copies: exact fp32 + f32r-rounded
                    src = ptg[:].rearrange("p (g v) -> p g v", v=128)[:, 0:gn, 0:TB]
                    d32 = xt_f32[:, PAD + g0 * TB:PAD + (g0 + gn) * TB].rearrange(
                        "p (g v) -> p g v", v=TB)
                    dhi = xt_hi[:, PAD + g0 * TB:PAD + (g0 + gn) * TB].rearrange(
                        "p (g v) -> p g v", v=TB)
                    nc.vector.tensor_copy(d32, src)
                    # f32r cast reads the SBUF copy, keeping the transpose
                    # PSUM at a single reader for fast recycling
                    nc.scalar.copy(dhi, d32)

                # mixed-precision FIR chunk: C_0 (taps 0..127) and C_1
                # (taps 1..255, contains the big h[1..127] subdiagonal) must
                # be exact fp32: f32r noises at ~2.4e-4 of each term's own
                # output scale.  C_2/C_3 terms (taps >= 129, ||h|| ~ 2e-3)
                # ride f32r at < 1e-6 cost.
                def fir_chunk(b0, n):
                    py = fir_ps.tile([U, 512], F32, tag="fir")
                    # fp32 matmuls first: they depend only on xt_f32, which is
                    # ready one copy earlier than the f32r cast (probe4: mixed
                    # fp32/f32r accumulation order does not affect accuracy)
                    mms = []
                    for s in (1, 0):
                        cs = slice(s * U, (s + 1) * U)
                        mms.append((cmat_raw[:, cs],
                                    xt_f32[:, PAD + b0 - s:PAD + b0 - s + n]))
                    for s in range(2, NTAP_BLK):
                        cs = slice(s * U, (s + 1) * U)
                        mms.append((cmat_hi[:, cs],
                                    xt_hi[:, PAD + b0 - s:PAD + b0 - s + n]))
                    for im, (lhsT, rhs) in enumerate(mms):
                        nc.tensor.matmul(py[:, :n], lhsT, rhs,
                                         start=(im == 0), stop=(im == len(mms) - 1))
                    nc.vector.tensor_copy(ytf[:, b0:b0 + n], py[:, :n])

                for g0, gn in tgroups:
                    in_group(g0, gn)
                for b0, n in CHUNKS:
                    fir_chunk(b0, n)

                # ---- stage C: 10 transposes back + one DMA out (on ACT)
                yn = yn_p.tile([TB, NT * U], F32)
                for gi, (g0, gn) in enumerate(TGROUPS):
                    pog = pog_ps.tile([TB, 512], F32, tag="pog")
                    for k in range(gn):
                        t = g0 + k
                        nc.tensor.transpose(
                            pog[:, 128 * k:128 * (k + 1)],
                            ytf[:, t * TB:(t + 1) * TB],
                            ident[:, :])
                    nc.scalar.copy(yn[:, g0 * U:(g0 + gn) * U], pog[:, 0:gn * 128])
                    if ch == CH - 1:
                        # last channel: per-group out-DMA on alternating
                        # queues shortens the serial HWDGE tail
                        eng = nc.sync if gi % 2 == 0 else nc.scalar
                        eng.dma_start(
                            y_r[ch, :, g0:g0 + gn],
                            yn[:, g0 * U:(g0 + gn) * U].rearrange(
                                "p (t u) -> p t u", u=U))
                if ch < CH - 1:
                    nc.scalar.dma_start(
                        y_r[ch], yn[:].rearrange("p (t u) -> p t u", u=U))

    nc.compile()
    return nc


_CACHE = {}


def _get_program():
    if "nc" not in _CACHE:
        _CACHE["nc"] = _build_program()
        _CACHE["cmat"] = _toeplitz_weights()
        _CACHE["ident"] = np.eye(U, dtype=np.float32)
    return _CACHE["nc"], _CACHE["cmat"], _CACHE["ident"]


def kernel(waveform: np.ndarray, _trace: bool = False) -> np.ndarray:
    nc, cmat, ident = _get_program()
    x = np.ascontiguousarray(np.asarray(waveform), dtype=np.float32)
    assert x.shape == (C_TOTAL, T_TOTAL)
    shards = x.reshape(N_CORES, CH, T_TOTAL)
    in_maps = [{"x": shards[c], "cmat": cmat, "ident": ident} for c in range(N_CORES)]
    if _trace:
        try:
            res = run_bass_kernel_spmd(
                nc, in_maps, core_ids=list(range(N_CORES)), trace=True)
            kernel.last_exec_time_ns = res.exec_time_ns
            return np.concatenate([r["y"] for r in res.results], axis=0)
        except Exception:
            kernel.last_exec_time_ns = None
    res = run_bass_kernel_spmd(nc, in_maps, core_ids=list(range(N_CORES)))
    return np.concatenate([r["y"] for r in res.results], axis=0)



# revision 3
# speedup vs baseline: 2.1075x; 2.1075x over previous
"""Trainium2 Bass kernel for nn_MicResponseAugment: HP(125Hz)+LP(6kHz) biquad
cascade over waveform [128, 160000] f32.

Algorithm: the biquad cascade is an LTI filter; its impulse response decays
like r^n (r = 0.9659), so a truncated causal FIR computed as block-Toeplitz
matmuls on the PE replaces the sequential IIR scan.  All FIR arithmetic is
bf16 (inputs, taps, outputs) with f32 PSUM accumulation: measured rel err
5.1e-3 against the f32 reference, dominated by bf16 quantization — well
under the 2e-2 gate — and 4x-8x cheaper on every engine than the fp32/f32r
mix.

Dataflow per channel (16 channels/core, data-parallel over 8 cores):
  1. one 640KB DMA in: xa[125 p, 10*128] f32 (block q = t*125+p of 128
     samples each; 512B-contiguous descriptors -> full 360 GB/s)
  2. 10 PE transposes (f32, 2 cyc/row) -> PSUM, batched 4/4/2 per bank
  3. DVE copy PSUM -> xt bf16 [128 k, 2+1250 q] (cast in the copy)
  4. FIR as X-stationary matmuls: stationary = stride-2 column windows of
     xt (125 block-pairs), moving = Toeplitz tap blocks C_s bf16 [128,128],
     s=0,1 (taps 0..255; coverage >= 129 taps/sample, truncation noise
     ~1e-2 of bf16 noise).  Output PSUM tile [125, 512] holds TWO groups of
     250 blocks: partition p carries 256 *consecutive* samples
  5. ACT copy PSUM -> yn bf16 [125, 1280] (cast)
  6. one 320KB DMA out (512B-contiguous bf16 runs -> full bandwidth);
     host upcasts bf16 -> f32

All 16 input DMAs are issued up front: the DMA engines are the roofline
(10.24MB in + 5.12MB out = 42.7us at 360 GB/s), and front-loading keeps the
PE fed back-to-back so it stays at its ramped clock (matmul cost doubles if
the pipeline restarts each channel).
"""

import numpy as np
from contextlib import ExitStack

import concourse.bacc as bacc
import concourse.bass as bass
import concourse.tile as tile
from concourse import mybir
from concourse.bass_utils import run_bass_kernel_spmd

# ---------------------------------------------------------------- constants
SR = 16000
HP_FREQ = 125.0
LP_FREQ = 6000.0
Q_FACT = 0.7071067811865476

N_CORES = 8
C_TOTAL = 128
T_TOTAL = 160000
CH = C_TOTAL // N_CORES          # 16 channels per core
U = 128                          # FIR block length
QB = T_TOTAL // U                # 1250 blocks per channel
TB = 125                         # blocks per transpose tile
NT = QB // TB                    # 10 transpose tiles per channel
PAD = 2                          # zero-history columns per channel
NTAP = 2                         # tap blocks: taps 0..255
NG = 5                           # output groups of 250 blocks (2 per psum bank)
GB = QB // NG                    # 250 blocks per output group
TGROUPS = [(0, 4), (4, 4), (8, 2)]

F32 = mybir.dt.float32
BF16 = mybir.dt.bfloat16


def _impulse_response(n: int) -> np.ndarray:
    """Cascade impulse response, float64 (from fp32-rounded coefficients)."""
    def coeffs(freq, highpass):
        w0 = 2.0 * np.pi * freq / SR
        cw, sw = np.cos(w0), np.sin(w0)
        al = sw / (2.0 * Q_FACT)
        if highpass:
            b = np.array([(1 + cw) / 2, -(1 + cw), (1 + cw) / 2])
        else:
            b = np.array([(1 - cw) / 2, (1 - cw), (1 - cw) / 2])
        a = np.array([1 + al, -2 * cw, 1 - al])
        b = (b / a[0]).astype(np.float32).astype(np.float64)
        a = (a / a[0]).astype(np.float32).astype(np.float64)
        return b, a

    def filt(x, b, a):
        y = np.zeros_like(x)
        for i in range(len(x)):
            acc = b[0] * x[i]
            if i >= 1:
                acc += b[1] * x[i - 1] - a[1] * y[i - 1]
            if i >= 2:
                acc += b[2] * x[i - 2] - a[2] * y[i - 2]
            y[i] = acc
        return y

    bh, ah = coeffs(HP_FREQ, True)
    bl, al = coeffs(LP_FREQ, False)
    x = np.zeros(n)
    x[0] = 1.0
    return filt(filt(x, bh, ah), bl, al)


def _toeplitz_weights() -> np.ndarray:
    """cmat[k, s*128 + i] = h[s*128 + i - k], shape [128, 256] bf16-valued f32.

    Stored in DRAM as uint16 (bf16 bit pattern) to avoid host-side ml_dtypes
    dependence in the DMA path; the dram tensor is declared bf16.
    """
    import ml_dtypes
    h = _impulse_response(NTAP * U)
    cmat = np.zeros((U, NTAP * U), dtype=np.float64)
    k = np.arange(U)[:, None]
    i = np.arange(U)[None, :]
    for s in range(NTAP):
        tau = s * U + i - k
        cmat[:, s * U:(s + 1) * U] = np.where(
            (tau >= 0) & (tau < NTAP * U), h[np.clip(tau, 0, NTAP * U - 1)], 0.0)
    return cmat.astype(np.float32).astype(ml_dtypes.bfloat16)


# ---------------------------------------------------------------- program
def _build_program():
    nc = bacc.Bacc("TRN2", target_bir_lowering=False, debug=False)
    x = nc.dram_tensor("x", [CH, T_TOTAL], F32, kind="ExternalInput")
    cmat_d = nc.dram_tensor("cmat", [U, NTAP * U], BF16, kind="ExternalInput")
    ident_d = nc.dram_tensor("ident", [U, U], F32, kind="ExternalInput")
    y = nc.dram_tensor("y", [CH, T_TOTAL], BF16, kind="ExternalOutput")

    # input view: block q = t*125 + p holds samples q*128 + u
    x_r = x.ap().rearrange("c (t p u) -> c p t u", t=NT, p=TB, u=U)
    # output view: partition p of group g holds samples (g*250+2p)*128 + i
    y_r = y.ap().rearrange("c (g p i) -> c p g i", g=NG, p=TB, i=2 * U)

    with tile.TileContext(nc) as tc:
        with ExitStack() as ctx:
            const_p = ctx.enter_context(tc.tile_pool(name="const", bufs=1))
            xa_p = ctx.enter_context(tc.tile_pool(name="xa", bufs=CH))
            xt_p = ctx.enter_context(tc.tile_pool(name="xt", bufs=3))
            yn_p = ctx.enter_context(tc.tile_pool(name="yn", bufs=4))
            ptg_ps = ctx.enter_context(tc.tile_pool(name="ptg", bufs=3, space="PSUM"))
            fir_ps = ctx.enter_context(tc.tile_pool(name="fir", bufs=3, space="PSUM"))

            ident = const_p.tile([U, U], F32)
            nc.sync.dma_start(ident[:], ident_d.ap()[:])
            cmat = const_p.tile([U, NTAP * U], BF16)
            nc.sync.dma_start(cmat[:], cmat_d.ap()[:])

            # front-load every channel's input DMA (DMA engines are the
            # roofline; keeps PE continuously fed and at ramped clock)
            xas = []
            for ch in range(CH):
                xa = xa_p.tile([TB, NT * U], F32)
                nc.sync.dma_start(
                    xa[:].rearrange("p (t u) -> p t u", u=U), x_r[ch])
                xas.append(xa)

            for ch in range(CH):
                xa = xas[ch]
                # ---- transpose to time-major: xt[k, PAD+q] bf16
                # +2 spare cols: the last stride-2 stationary window's slice
                # extends past q=1249 (only even offsets are addressed)
                xt = xt_p.tile([U, PAD + QB + 2], BF16)
                nc.vector.memset(xt[:, 0:PAD], 0)
                for g0, gn in TGROUPS:
                    ptg = ptg_ps.tile([U, 512], F32, tag="ptg")
                    for t in range(gn):
                        nc.tensor.transpose(
                            ptg[:, 128 * t:128 * t + TB],
                            xa[:, (g0 + t) * U:(g0 + t + 1) * U],
                            ident[:TB, :TB])
                    src = ptg[:].rearrange("p (g v) -> p g v", v=128)[:, 0:gn, 0:TB]
                    dst = xt[:, PAD + g0 * TB:PAD + (g0 + gn) * TB].rearrange(
                        "p (g v) -> p g v", v=TB)
                    nc.vector.tensor_copy(dst, src)

                # ---- FIR: X-stationary paired-block matmuls
                yn = yn_p.tile([TB, QB * U // TB], BF16)  # [125, 1280]
                for g0, gcnt in ((0, 2), (2, 2), (4, 1)):
                    py = fir_ps.tile([TB, 512], F32, tag="fir")
                    for j in range(gcnt):
                        b0 = (g0 + j) * GB
                        for h in (0, 1):
                            out_ap = py[:, j * 256 + h * U:j * 256 + (h + 1) * U]
                            for s in range(NTAP):
                                c0 = PAD + b0 + h - s
                                lhsT = xt[:, c0:c0 + 2 * TB].rearrange(
                                    "k (p two) -> k two p", two=2)[:, 0, :]
                                nc.tensor.matmul(
                                    out_ap, lhsT, cmat[:, s * U:(s + 1) * U],
                                    start=(s == 0), stop=(s == NTAP - 1))
                    nc.scalar.copy(
                        yn[:, g0 * 256:(g0 + gcnt) * 256], py[:, :gcnt * 256])

                nc.sync.dma_start(
                    y_r[ch], yn[:].rearrange("p (g i) -> p g i", i=2 * U))

    nc.compile()
    return nc


_CACHE = {}


def _get_program():
    if "nc" not in _CACHE:
        _CACHE["nc"] = _build_program()
        _CACHE["cmat"] = _toeplitz_weights()
        _CACHE["ident"] = np.eye(U, dtype=np.float32)
    return _CACHE["nc"], _CACHE["cmat"], _CACHE["ident"]


def kernel(waveform: np.ndarray, _trace: bool = False) -> np.ndarray:
    nc, cmat, ident = _get_program()
    x = np.ascontiguousarray(np.asarray(waveform), dtype=np.float32)
    assert x.shape == (C_TOTAL, T_TOTAL)
    shards = x.reshape(N_CORES, CH, T_TOTAL)
    in_maps = [{"x": shards[c], "cmat": cmat, "ident": ident} for c in range(N_CORES)]
    if _trace:
        try:
            res = run_bass_kernel_spmd(
                nc, in_maps, core_ids=list(range(N_CORES)), trace=True)
            kernel.last_exec_time_ns = res.exec_time_ns
            return np.concatenate(
                [np.asarray(r["y"]).astype(np.float32) for r in res.results], axis=0)
        except Exception:
            kernel.last_exec_time_ns = None
    res = run_bass_kernel_spmd(nc, in_maps, core_ids=list(range(N_CORES)))
    return np.concatenate(
        [np.asarray(r["y"]).astype(np.float32) for r in res.results], axis=0)


# revision 5
# speedup vs baseline: 2.6963x; 1.2794x over previous
"""Trainium2 Bass kernel for nn_MicResponseAugment: HP(125Hz)+LP(6kHz) biquad
cascade over waveform [128, 160000] f32.

Algorithm: the biquad cascade is an LTI filter; its impulse response decays
like r^n (r = 0.9659), so a truncated causal FIR computed as block-Toeplitz
matmuls on the PE replaces the sequential IIR scan.  All FIR arithmetic is
bf16 (inputs, taps, outputs) with f32 PSUM accumulation: measured rel err
5.1e-3 against the f32 reference, dominated by bf16 quantization — well
under the 2e-2 gate — and 4x-8x cheaper on every engine than the fp32/f32r
mix.

Dataflow per channel (16 channels/core, data-parallel over 8 cores):
  1. one 640KB DMA in: xa[125 p, 10*128] f32 (block q = t*125+p of 128
     samples each; 512B-contiguous descriptors -> full 360 GB/s)
  2. 10 PE transposes (f32, 2 cyc/row) -> PSUM, batched 4/4/2 per bank
  3. DVE copy PSUM -> xt bf16 [128 k, 2+1250 q] (cast in the copy)
  4. FIR as X-stationary matmuls: stationary = stride-2 column windows of
     xt (125 block-pairs), moving = Toeplitz tap blocks C_s bf16 [128,128],
     s=0,1 (taps 0..255; coverage >= 129 taps/sample, truncation noise
     ~1e-2 of bf16 noise).  Output PSUM tile [125, 512] holds TWO groups of
     250 blocks: partition p carries 256 *consecutive* samples
  5. ACT copy PSUM -> yn bf16 [125, 1280] (cast)
  6. one 320KB DMA out (512B-contiguous bf16 runs -> full bandwidth);
     host upcasts bf16 -> f32

All 16 input DMAs are issued up front: the DMA engines are the roofline
(10.24MB in + 5.12MB out = 42.7us at 360 GB/s), and front-loading keeps the
PE fed back-to-back so it stays at its ramped clock (matmul cost doubles if
the pipeline restarts each channel).
"""

import numpy as np
from contextlib import ExitStack

import concourse.bacc as bacc
import concourse.bass as bass
import concourse.tile as tile
from concourse import mybir
from concourse.bass_utils import run_bass_kernel_spmd

# ---------------------------------------------------------------- constants
SR = 16000
HP_FREQ = 125.0
LP_FREQ = 6000.0
Q_FACT = 0.7071067811865476

N_CORES = 8
C_TOTAL = 128
T_TOTAL = 160000
CH = C_TOTAL // N_CORES          # 16 channels per core
U = 128                          # FIR block length
QB = T_TOTAL // U                # 1250 blocks per channel
TB = 125                         # blocks per transpose tile
NT = QB // TB                    # 10 transpose tiles per channel
PAD = 2                          # zero-history columns per channel
NTAP = 2                         # tap blocks: taps 0..255
NG = 5                           # output groups of 250 blocks (2 per psum bank)
GB = QB // NG                    # 250 blocks per output group
TGROUPS = [(0, 4), (4, 4), (8, 2)]

F32 = mybir.dt.float32
BF16 = mybir.dt.bfloat16


def _impulse_response(n: int) -> np.ndarray:
    """Cascade impulse response, float64 (from fp32-rounded coefficients)."""
    def coeffs(freq, highpass):
        w0 = 2.0 * np.pi * freq / SR
        cw, sw = np.cos(w0), np.sin(w0)
        al = sw / (2.0 * Q_FACT)
        if highpass:
            b = np.array([(1 + cw) / 2, -(1 + cw), (1 + cw) / 2])
        else:
            b = np.array([(1 - cw) / 2, (1 - cw), (1 - cw) / 2])
        a = np.array([1 + al, -2 * cw, 1 - al])
        b = (b / a[0]).astype(np.float32).astype(np.float64)
        a = (a / a[0]).astype(np.float32).astype(np.float64)
        return b, a

    def filt(x, b, a):
        y = np.zeros_like(x)
        for i in range(len(x)):
            acc = b[0] * x[i]
            if i >= 1:
                acc += b[1] * x[i - 1] - a[1] * y[i - 1]
            if i >= 2:
                acc += b[2] * x[i - 2] - a[2] * y[i - 2]
            y[i] = acc
        return y

    bh, ah = coeffs(HP_FREQ, True)
    bl, al = coeffs(LP_FREQ, False)
    x = np.zeros(n)
    x[0] = 1.0
    return filt(filt(x, bh, ah), bl, al)


def _toeplitz_weights() -> np.ndarray:
    """cmat[k, s*128 + i] = h[s*128 + i - k], shape [128, 256] bf16-valued f32.

    Stored in DRAM as uint16 (bf16 bit pattern) to avoid host-side ml_dtypes
    dependence in the DMA path; the dram tensor is declared bf16.
    """
    import ml_dtypes
    h = _impulse_response(NTAP * U)
    cmat = np.zeros((U, NTAP * U), dtype=np.float64)
    k = np.arange(U)[:, None]
    i = np.arange(U)[None, :]
    for s in range(NTAP):
        tau = s * U + i - k
        cmat[:, s * U:(s + 1) * U] = np.where(
            (tau >= 0) & (tau < NTAP * U), h[np.clip(tau, 0, NTAP * U - 1)], 0.0)
    return cmat.astype(np.float32).astype(ml_dtypes.bfloat16)


# ---------------------------------------------------------------- program
def _build_program():
    nc = bacc.Bacc("TRN2", target_bir_lowering=False, debug=False)
    x = nc.dram_tensor("x", [CH, T_TOTAL], F32, kind="ExternalInput")
    cmat_d = nc.dram_tensor("cmat", [U, NTAP * U], BF16, kind="ExternalInput")
    ident_d = nc.dram_tensor("ident", [U, U], F32, kind="ExternalInput")
    y = nc.dram_tensor("y", [CH, T_TOTAL], BF16, kind="ExternalOutput")

    # input view: block q = t*125 + p holds samples q*128 + u
    x_r = x.ap().rearrange("c (t p u) -> c p t u", t=NT, p=TB, u=U)
    # output view: partition p of group g holds samples (g*250+2p)*128 + i
    y_r = y.ap().rearrange("c (g p i) -> c p g i", g=NG, p=TB, i=2 * U)

    with tile.TileContext(nc) as tc:
        with ExitStack() as ctx:
            const_p = ctx.enter_context(tc.tile_pool(name="const", bufs=1))
            xa_p = ctx.enter_context(tc.tile_pool(name="xa", bufs=CH))
            xt_p = ctx.enter_context(tc.tile_pool(name="xt", bufs=3))
            # all yn bufs resident: out-DMAs queue behind the 16 front-loaded
            # input DMAs on the DMA engines, so compute must never block on a
            # yn buffer waiting for an out-DMA to retire it
            yn_p = ctx.enter_context(tc.tile_pool(name="yn", bufs=CH))
            ptg_ps = ctx.enter_context(tc.tile_pool(name="ptg", bufs=3, space="PSUM"))
            fir_ps = ctx.enter_context(tc.tile_pool(name="fir", bufs=3, space="PSUM"))

            ident = const_p.tile([U, U], F32)
            nc.sync.dma_start(ident[:], ident_d.ap()[:])
            cmat = const_p.tile([U, NTAP * U], BF16)
            nc.sync.dma_start(cmat[:], cmat_d.ap()[:])

            # front-load every channel's input DMA (DMA engines are the
            # roofline; keeps PE continuously fed and at ramped clock)
            xas = []
            for ch in range(CH):
                xa = xa_p.tile([TB, NT * U], F32)
                nc.sync.dma_start(
                    xa[:].rearrange("p (t u) -> p t u", u=U), x_r[ch])
                xas.append(xa)

            def emit_transpose(ch):
                # ---- transpose to time-major: xt[k, PAD+q] bf16
                # +2 spare cols: the last stride-2 stationary window's slice
                # extends past q=1249 (only even offsets are addressed)
                xa = xas[ch]
                xt = xt_p.tile([U, PAD + QB + 2], BF16)
                nc.vector.memset(xt[:, 0:PAD], 0)
                for g0, gn in TGROUPS:
                    ptg = ptg_ps.tile([U, 512], F32, tag="ptg")
                    for t in range(gn):
                        nc.tensor.transpose(
                            ptg[:, 128 * t:128 * t + TB],
                            xa[:, (g0 + t) * U:(g0 + t + 1) * U],
                            ident[:TB, :TB])
                    src = ptg[:].rearrange("p (g v) -> p g v", v=128)[:, 0:gn, 0:TB]
                    dst = xt[:, PAD + g0 * TB:PAD + (g0 + gn) * TB].rearrange(
                        "p (g v) -> p g v", v=TB)
                    nc.vector.tensor_copy(dst, src)
                return xt

            def emit_fir(ch, xt):
                # ---- FIR: X-stationary paired-block matmuls
                yn = yn_p.tile([TB, QB * U // TB], BF16)  # [125, 1280]
                for g0, gcnt in ((0, 2), (2, 2), (4, 1)):
                    py = fir_ps.tile([TB, 512], F32, tag="fir")
                    for j in range(gcnt):
                        b0 = (g0 + j) * GB
                        for h in (0, 1):
                            out_ap = py[:, j * 256 + h * U:j * 256 + (h + 1) * U]
                            for s in range(NTAP):
                                c0 = PAD + b0 + h - s
                                lhsT = xt[:, c0:c0 + 2 * TB].rearrange(
                                    "k (p two) -> k two p", two=2)[:, 0, :]
                                nc.tensor.matmul(
                                    out_ap, lhsT, cmat[:, s * U:(s + 1) * U],
                                    start=(s == 0), stop=(s == NTAP - 1))
                    nc.scalar.copy(
                        yn[:, g0 * 256:(g0 + gcnt) * 256], py[:, :gcnt * 256])

                nc.sync.dma_start(
                    y_r[ch], yn[:].rearrange("p (g i) -> p g i", i=2 * U))

            # software-pipeline the PE stream: channel ch+1's transposes are
            # emitted before channel ch's FIR so the PE never sits out the
            # DVE copy that completes ch's xt
            xts = {}
            for ch in range(CH):
                xts[ch] = emit_transpose(ch)
                if ch > 0:
                    emit_fir(ch - 1, xts.pop(ch - 1))
            emit_fir(CH - 1, xts.pop(CH - 1))

    nc.compile()
    return nc


_CACHE = {}


def _get_program():
    if "nc" not in _CACHE:
        _CACHE["nc"] = _build_program()
        _CACHE["cmat"] = _toeplitz_weights()
        _CACHE["ident"] = np.eye(U, dtype=np.float32)
    return _CACHE["nc"], _CACHE["cmat"], _CACHE["ident"]


def kernel(waveform: np.ndarray, _trace: bool = False) -> np.ndarray:
    nc, cmat, ident = _get_program()
    x = np.ascontiguousarray(np.asarray(waveform), dtype=np.float32)
    assert x.shape == (C_TOTAL, T_TOTAL)
    shards = x.reshape(N_CORES, CH, T_TOTAL)
    in_maps = [{"x": shards[c], "cmat": cmat, "ident": ident} for c in range(N_CORES)]
    if _trace:
        try:
            res = run_bass_kernel_spmd(
                nc, in_maps, core_ids=list(range(N_CORES)), trace=True)
            kernel.last_exec_time_ns = res.exec_time_ns
            return np.concatenate(
                [np.asarray(r["y"]).astype(np.float32) for r in res.results], axis=0)
        except Exception:
            kernel.last_exec_time_ns = None
    res = run_bass_kernel_spmd(nc, in_maps, core_ids=list(range(N_CORES)))
    return np.concatenate(
        [np.asarray(r["y"]).astype(np.float32) for r in res.results], axis=0)


# revision 14
# speedup vs baseline: 2.7689x; 1.0269x over previous
"""Trainium2 Bass kernel for nn_MicResponseAugment: HP(125Hz)+LP(6kHz) biquad
cascade over waveform [128, 160000] f32.

Algorithm: the biquad cascade is an LTI filter; its impulse response decays
like r^n (r = 0.9659), so a truncated causal FIR computed as block-Toeplitz
matmuls on the PE replaces the sequential IIR scan.  All FIR arithmetic is
bf16 (inputs, taps, outputs) with f32 PSUM accumulation: measured rel err
5.1e-3 against the f32 reference, dominated by bf16 quantization — well
under the 2e-2 gate — and 4x-8x cheaper on every engine than the fp32/f32r
mix.

Dataflow per channel (16 channels/core, data-parallel over 8 cores):
  1. one 640KB DMA in: xa[125 p, 10*128] f32 (block q = t*125+p of 128
     samples each; 512B-contiguous descriptors -> full 360 GB/s)
  2. 10 PE transposes (f32, 2 cyc/row) -> PSUM, batched 4/4/2 per bank
  3. DVE copy PSUM -> xt bf16 [128 k, 2+1250 q] (cast in the copy)
  4. FIR as X-stationary matmuls: stationary = stride-2 column windows of
     xt (125 block-pairs), moving = Toeplitz tap blocks C_s bf16 [128,128],
     s=0,1 (taps 0..255; coverage >= 129 taps/sample, truncation noise
     ~1e-2 of bf16 noise).  Output PSUM tile [125, 512] holds TWO groups of
     250 blocks: partition p carries 256 *consecutive* samples
  5. ACT copy PSUM -> yn bf16 [125, 1280] (cast)
  6. one 320KB DMA out (512B-contiguous bf16 runs -> full bandwidth);
     host upcasts bf16 -> f32

All 16 input DMAs are issued up front: the DMA engines are the roofline
(10.24MB in + 5.12MB out = 42.7us at 360 GB/s), and front-loading keeps the
PE fed back-to-back so it stays at its ramped clock (matmul cost doubles if
the pipeline restarts each channel).
"""

import numpy as np
from contextlib import ExitStack

import concourse.bacc as bacc
import concourse.bass as bass
import concourse.tile as tile
from concourse import mybir
from concourse.bass_utils import run_bass_kernel_spmd

# ---------------------------------------------------------------- constants
SR = 16000
HP_FREQ = 125.0
LP_FREQ = 6000.0
Q_FACT = 0.7071067811865476

N_CORES = 8
C_TOTAL = 128
T_TOTAL = 160000
CH = C_TOTAL // N_CORES          # 16 channels per core
U = 128                          # FIR block length
QB = T_TOTAL // U                # 1250 blocks per channel
TB = 125                         # blocks per transpose tile
NT = QB // TB                    # 10 transpose tiles per channel
PAD = 2                          # zero-history columns per channel
NTAP = 2                         # tap blocks: taps 0..255
NG = 5                           # output groups of 250 blocks (2 per psum bank)
GB = QB // NG                    # 250 blocks per output group
TGROUPS = [(0, 4), (4, 4), (8, 2)]

F32 = mybir.dt.float32
BF16 = mybir.dt.bfloat16


def _impulse_response(n: int) -> np.ndarray:
    """Cascade impulse response, float64 (from fp32-rounded coefficients)."""
    def coeffs(freq, highpass):
        w0 = 2.0 * np.pi * freq / SR
        cw, sw = np.cos(w0), np.sin(w0)
        al = sw / (2.0 * Q_FACT)
        if highpass:
            b = np.array([(1 + cw) / 2, -(1 + cw), (1 + cw) / 2])
        else:
            b = np.array([(1 - cw) / 2, (1 - cw), (1 - cw) / 2])
        a = np.array([1 + al, -2 * cw, 1 - al])
        b = (b / a[0]).astype(np.float32).astype(np.float64)
        a = (a / a[0]).astype(np.float32).astype(np.float64)
        return b, a

    def filt(x, b, a):
        y = np.zeros_like(x)
        for i in range(len(x)):
            acc = b[0] * x[i]
            if i >= 1:
                acc += b[1] * x[i - 1] - a[1] * y[i - 1]
            if i >= 2:
                acc += b[2] * x[i - 2] - a[2] * y[i - 2]
            y[i] = acc
        return y

    bh, ah = coeffs(HP_FREQ, True)
    bl, al = coeffs(LP_FREQ, False)
    x = np.zeros(n)
    x[0] = 1.0
    return filt(filt(x, bh, ah), bl, al)


def _toeplitz_weights() -> np.ndarray:
    """cmat[k, s*128 + i] = h[s*128 + i - k], shape [128, 256] bf16-valued f32.

    Stored in DRAM as uint16 (bf16 bit pattern) to avoid host-side ml_dtypes
    dependence in the DMA path; the dram tensor is declared bf16.
    """
    import ml_dtypes
    h = _impulse_response(NTAP * U)
    cmat = np.zeros((U, NTAP * U), dtype=np.float64)
    k = np.arange(U)[:, None]
    i = np.arange(U)[None, :]
    for s in range(NTAP):
        tau = s * U + i - k
        cmat[:, s * U:(s + 1) * U] = np.where(
            (tau >= 0) & (tau < NTAP * U), h[np.clip(tau, 0, NTAP * U - 1)], 0.0)
    return cmat.astype(np.float32).astype(ml_dtypes.bfloat16)


# ---------------------------------------------------------------- program
def _build_program():
    nc = bacc.Bacc("TRN2", target_bir_lowering=False, debug=False)
    x = nc.dram_tensor("x", [CH, T_TOTAL], F32, kind="ExternalInput")
    cmat_d = nc.dram_tensor("cmat", [U, NTAP * U], BF16, kind="ExternalInput")
    ident_d = nc.dram_tensor("ident", [U, U], F32, kind="ExternalInput")
    y = nc.dram_tensor("y", [CH, T_TOTAL], BF16, kind="ExternalOutput")

    # input view: block q = t*125 + p holds samples q*128 + u
    x_r = x.ap().rearrange("c (t p u) -> c p t u", t=NT, p=TB, u=U)
    # output view: partition p of group g holds samples (g*250+2p)*128 + i
    y_r = y.ap().rearrange("c (g p i) -> c p g i", g=NG, p=TB, i=2 * U)

    with tile.TileContext(nc) as tc:
        with ExitStack() as ctx:
            const_p = ctx.enter_context(tc.tile_pool(name="const", bufs=1))
            xa_p = ctx.enter_context(tc.tile_pool(name="xa", bufs=CH))
            xab_p = ctx.enter_context(tc.tile_pool(name="xab", bufs=3))
            xt_p = ctx.enter_context(tc.tile_pool(name="xt", bufs=3))
            # all yn bufs resident: out-DMAs queue behind the 16 front-loaded
            # input DMAs on the DMA engines, so compute must never block on a
            # yn buffer waiting for an out-DMA to retire it
            yn_p = ctx.enter_context(tc.tile_pool(name="yn", bufs=CH))
            ptg_ps = ctx.enter_context(tc.tile_pool(name="ptg", bufs=3, space="PSUM"))
            fir_ps = ctx.enter_context(tc.tile_pool(name="fir", bufs=3, space="PSUM"))

            # front-load every channel's input DMA (DMA engines are the
            # roofline; keeps PE continuously fed and at ramped clock).
            # Channel 0 goes first so the pipeline's head starts at the
            # earliest possible grant; the tiny const DMAs slot in behind it.
            ident = const_p.tile([U, U], F32)
            cmat = const_p.tile([U, NTAP * U], BF16)
            xas = []
            for ch in range(CH):
                xa = xa_p.tile([TB, NT * U], F32)
                nc.sync.dma_start(
                    xa[:].rearrange("p (t u) -> p t u", u=U), x_r[ch])
                xas.append(xa)
                if ch == 0:
                    nc.sync.dma_start(ident[:], ident_d.ap()[:])
                    nc.sync.dma_start(cmat[:], cmat_d.ap()[:])

            def emit_transpose(ch):
                # ---- transpose to time-major: xt[k, PAD+q] bf16
                # +2 spare cols: the last stride-2 stationary window's slice
                # extends past q=1249 (only even offsets are addressed)
                # Pool (otherwise idle) pre-casts f32 -> bf16 so the PE
                # transposes run at 1 cyc/row instead of 2
                xa = xas[ch]
                xt = xt_p.tile([U, PAD + QB + 2], BF16)
                nc.vector.memset(xt[:, 0:PAD], 0)
                for g0, gn in TGROUPS:
                    ptg = ptg_ps.tile([U, 512], F32, tag="ptg")
                    for t in range(gn):
                        nc.tensor.transpose(
                            ptg[:, 128 * t:128 * t + TB],
                            xa[:, (g0 + t) * U:(g0 + t + 1) * U],
                            ident[:TB, :TB])
                    src = ptg[:].rearrange("p (g v) -> p g v", v=128)[:, 0:gn, 0:TB]
                    dst = xt[:, PAD + g0 * TB:PAD + (g0 + gn) * TB].rearrange(
                        "p (g v) -> p g v", v=TB)
                    nc.vector.tensor_copy(dst, src)
                return xt

            def emit_fir(ch, xt):
                # ---- FIR: X-stationary paired-block matmuls
                yn = yn_p.tile([TB, QB * U // TB], BF16)  # [125, 1280]
                last = ch == CH - 1
                # last channel: spread the PSUM->yn copies over three engines
                # and split the out-DMA, shortening the serial tail after the
                # final matmul (everything else has drained by then)
                # (gpsimd cannot read PSUM, so the third engine is ACT again)
                copy_engines = (
                    (nc.scalar, nc.vector, nc.scalar) if last
                    else (nc.scalar, nc.scalar, nc.scalar))
                for gi, (g0, gcnt) in enumerate(((0, 2), (2, 2), (4, 1))):
                    py = fir_ps.tile([TB, 512], F32, tag="fir")
                    for j in range(gcnt):
                        b0 = (g0 + j) * GB
                        for h in (0, 1):
                            out_ap = py[:, j * 256 + h * U:j * 256 + (h + 1) * U]
                            for s in range(NTAP):
                                c0 = PAD + b0 + h - s
                                lhsT = xt[:, c0:c0 + 2 * TB].rearrange(
                                    "k (p two) -> k two p", two=2)[:, 0, :]
                                nc.tensor.matmul(
                                    out_ap, lhsT, cmat[:, s * U:(s + 1) * U],
                                    start=(s == 0), stop=(s == NTAP - 1))
                    eng = copy_engines[gi]
                    if eng is nc.scalar:
                        eng.copy(
                            yn[:, g0 * 256:(g0 + gcnt) * 256], py[:, :gcnt * 256])
                    else:
                        eng.tensor_copy(
                            yn[:, g0 * 256:(g0 + gcnt) * 256], py[:, :gcnt * 256])
                    if last:
                        nc.sync.dma_start(
                            y_r[ch][:, g0:g0 + gcnt],
                            yn[:, g0 * 256:(g0 + gcnt) * 256].rearrange(
                                "p (g i) -> p g i", i=2 * U))
                if not last:
                    nc.sync.dma_start(
                        y_r[ch], yn[:].rearrange("p (g i) -> p g i", i=2 * U))

            # software-pipeline the PE stream: channel ch+1's transposes are
            # emitted before channel ch's FIR so the PE never sits out the
            # DVE copy that completes ch's xt
            xts = {}
            for ch in range(CH):
                xts[ch] = emit_transpose(ch)
                if ch > 0:
                    emit_fir(ch - 1, xts.pop(ch - 1))
            emit_fir(CH - 1, xts.pop(CH - 1))

    nc.compile()
    return nc


_CACHE = {}


def _get_program():
    if "nc" not in _CACHE:
        _CACHE["nc"] = _build_program()
        _CACHE["cmat"] = _toeplitz_weights()
        _CACHE["ident"] = np.eye(U, dtype=np.float32)
    return _CACHE["nc"], _CACHE["cmat"], _CACHE["ident"]


def kernel(waveform: np.ndarray, _trace: bool = False) -> np.ndarray:
    nc, cmat, ident = _get_program()
    x = np.ascontiguousarray(np.asarray(waveform), dtype=np.float32)
    assert x.shape == (C_TOTAL, T_TOTAL)
    shards = x.reshape(N_CORES, CH, T_TOTAL)
    in_maps = [{"x": shards[c], "cmat": cmat, "ident": ident} for c in range(N_CORES)]
    if _trace:
        try:
            res = run_bass_kernel_spmd(
                nc, in_maps, core_ids=list(range(N_CORES)), trace=True)
            kernel.last_exec_time_ns = res.exec_time_ns
            return np.concatenate(
                [np.asarray(r["y"]).astype(np.float32) for r in res.results], axis=0)
        except Exception:
            kernel.last_exec_time_ns = None
    res = run_bass_kernel_spmd(nc, in_maps, core_ids=list(range(N_CORES)))
    return np.concatenate(
        [np.asarray(r["y"]).astype(np.float32) for r in res.results], axis=0)


# revision 16
# speedup vs baseline: 2.8211x; 1.0188x over previous
"""Trainium2 Bass kernel for nn_MicResponseAugment: HP(125Hz)+LP(6kHz) biquad
cascade over waveform [128, 160000] f32.

Algorithm: the biquad cascade is an LTI filter; its impulse response decays
like r^n (r = 0.9659), so a truncated causal FIR computed as block-Toeplitz
matmuls on the PE replaces the sequential IIR scan.  All FIR arithmetic is
bf16 (inputs, taps, outputs) with f32 PSUM accumulation: measured rel err
5.1e-3 against the f32 reference, dominated by bf16 quantization — well
under the 2e-2 gate — and 4x-8x cheaper on every engine than the fp32/f32r
mix.

Dataflow per channel (16 channels/core, data-parallel over 8 cores):
  1. one 640KB DMA in: xa[125 p, 10*128] f32 (block q = t*125+p of 128
     samples each; 512B-contiguous descriptors -> full 360 GB/s)
  2. 10 PE transposes (f32, 2 cyc/row) -> PSUM, batched 4/4/2 per bank
  3. DVE copy PSUM -> xt bf16 [128 k, 2+1250 q] (cast in the copy)
  4. FIR as X-stationary matmuls: stationary = stride-2 column windows of
     xt (125 block-pairs), moving = Toeplitz tap blocks C_s bf16 [128,128],
     s=0,1 (taps 0..255; coverage >= 129 taps/sample, truncation noise
     ~1e-2 of bf16 noise).  Output PSUM tile [125, 512] holds TWO groups of
     250 blocks: partition p carries 256 *consecutive* samples
  5. ACT copy PSUM -> yn bf16 [125, 1280] (cast)
  6. one 320KB DMA out (512B-contiguous bf16 runs -> full bandwidth);
     host upcasts bf16 -> f32

All 16 input DMAs are issued up front: the DMA engines are the roofline
(10.24MB in + 5.12MB out = 42.7us at 360 GB/s), and front-loading keeps the
PE fed back-to-back so it stays at its ramped clock (matmul cost doubles if
the pipeline restarts each channel).
"""

import numpy as np
from contextlib import ExitStack

import concourse.bacc as bacc
import concourse.bass as bass
import concourse.tile as tile
from concourse import mybir
from concourse.bass_utils import run_bass_kernel_spmd

# ---------------------------------------------------------------- constants
SR = 16000
HP_FREQ = 125.0
LP_FREQ = 6000.0
Q_FACT = 0.7071067811865476

N_CORES = 8
C_TOTAL = 128
T_TOTAL = 160000
CH = C_TOTAL // N_CORES          # 16 channels per core
U = 128                          # FIR block length
QB = T_TOTAL // U                # 1250 blocks per channel
TB = 125                         # blocks per transpose tile
NT = QB // TB                    # 10 transpose tiles per channel
PAD = 2                          # zero-history columns per channel
NTAP = 2                         # tap blocks: taps 0..255
NG = 5                           # output groups of 250 blocks (2 per psum bank)
GB = QB // NG                    # 250 blocks per output group
TGROUPS = [(0, 4), (4, 4), (8, 2)]

F32 = mybir.dt.float32
BF16 = mybir.dt.bfloat16


def _impulse_response(n: int) -> np.ndarray:
    """Cascade impulse response, float64 (from fp32-rounded coefficients)."""
    def coeffs(freq, highpass):
        w0 = 2.0 * np.pi * freq / SR
        cw, sw = np.cos(w0), np.sin(w0)
        al = sw / (2.0 * Q_FACT)
        if highpass:
            b = np.array([(1 + cw) / 2, -(1 + cw), (1 + cw) / 2])
        else:
            b = np.array([(1 - cw) / 2, (1 - cw), (1 - cw) / 2])
        a = np.array([1 + al, -2 * cw, 1 - al])
        b = (b / a[0]).astype(np.float32).astype(np.float64)
        a = (a / a[0]).astype(np.float32).astype(np.float64)
        return b, a

    def filt(x, b, a):
        y = np.zeros_like(x)
        for i in range(len(x)):
            acc = b[0] * x[i]
            if i >= 1:
                acc += b[1] * x[i - 1] - a[1] * y[i - 1]
            if i >= 2:
                acc += b[2] * x[i - 2] - a[2] * y[i - 2]
            y[i] = acc
        return y

    bh, ah = coeffs(HP_FREQ, True)
    bl, al = coeffs(LP_FREQ, False)
    x = np.zeros(n)
    x[0] = 1.0
    return filt(filt(x, bh, ah), bl, al)


def _toeplitz_weights() -> np.ndarray:
    """cmat[k, s*128 + i] = h[s*128 + i - k], shape [128, 256] bf16-valued f32.

    Stored in DRAM as uint16 (bf16 bit pattern) to avoid host-side ml_dtypes
    dependence in the DMA path; the dram tensor is declared bf16.
    """
    import ml_dtypes
    h = _impulse_response(NTAP * U)
    cmat = np.zeros((U, NTAP * U), dtype=np.float64)
    k = np.arange(U)[:, None]
    i = np.arange(U)[None, :]
    for s in range(NTAP):
        tau = s * U + i - k
        cmat[:, s * U:(s + 1) * U] = np.where(
            (tau >= 0) & (tau < NTAP * U), h[np.clip(tau, 0, NTAP * U - 1)], 0.0)
    return cmat.astype(np.float32).astype(ml_dtypes.bfloat16)


# ---------------------------------------------------------------- program
def _build_program():
    nc = bacc.Bacc("TRN2", target_bir_lowering=False, debug=False)
    x = nc.dram_tensor("x", [CH, T_TOTAL], F32, kind="ExternalInput")
    cmat_d = nc.dram_tensor("cmat", [U, NTAP * U], BF16, kind="ExternalInput")
    ident_d = nc.dram_tensor("ident", [U, U], BF16, kind="ExternalInput")
    y = nc.dram_tensor("y", [CH, T_TOTAL], BF16, kind="ExternalOutput")

    # input view: block q = t*125 + p holds samples q*128 + u
    x_r = x.ap().rearrange("c (t p u) -> c p t u", t=NT, p=TB, u=U)
    # output view: partition p of group g holds samples (g*250+2p)*128 + i
    y_r = y.ap().rearrange("c (g p i) -> c p g i", g=NG, p=TB, i=2 * U)

    with tile.TileContext(nc) as tc:
        with ExitStack() as ctx:
            const_p = ctx.enter_context(tc.tile_pool(name="const", bufs=1))
            xa_p = ctx.enter_context(tc.tile_pool(name="xa", bufs=CH))
            xab_p = ctx.enter_context(tc.tile_pool(name="xab", bufs=3))
            xt_p = ctx.enter_context(tc.tile_pool(name="xt", bufs=3))
            # all yn bufs resident: out-DMAs queue behind the 16 front-loaded
            # input DMAs on the DMA engines, so compute must never block on a
            # yn buffer waiting for an out-DMA to retire it
            yn_p = ctx.enter_context(tc.tile_pool(name="yn", bufs=CH))
            ptg_ps = ctx.enter_context(tc.tile_pool(name="ptg", bufs=3, space="PSUM"))
            fir_ps = ctx.enter_context(tc.tile_pool(name="fir", bufs=3, space="PSUM"))

            # front-load every channel's input DMA (DMA engines are the
            # roofline; keeps PE continuously fed and at ramped clock).
            # Channel 0 goes first so the pipeline's head starts at the
            # earliest possible grant; the tiny const DMAs slot in behind it.
            ident = const_p.tile([U, U], BF16)
            cmat = const_p.tile([U, NTAP * U], BF16)
            xas = []
            for ch in range(CH):
                xa = xa_p.tile([TB, NT * U], F32)
                nc.sync.dma_start(
                    xa[:].rearrange("p (t u) -> p t u", u=U), x_r[ch])
                xas.append(xa)
                if ch == 0:
                    nc.sync.dma_start(ident[:], ident_d.ap()[:])
                    nc.sync.dma_start(cmat[:], cmat_d.ap()[:])

            def emit_transpose(ch):
                # ---- transpose to time-major: xt[k, PAD+q] bf16
                # +2 spare cols: the last stride-2 stationary window's slice
                # extends past q=1249 (only even offsets are addressed)
                # Pool (otherwise idle) pre-casts f32 -> bf16 so the PE
                # transposes run at 1 cyc/row instead of 2
                xa = xas[ch]
                xab = xab_p.tile([TB, NT * U], BF16)
                nc.gpsimd.tensor_copy(xab[:], xa[:])
                xt = xt_p.tile([U, PAD + QB + 2], BF16)
                nc.vector.memset(xt[:, 0:PAD], 0)
                for g0, gn in TGROUPS:
                    ptg = ptg_ps.tile([U, 512], BF16, tag="ptg", padded_shape=[U, 1024])
                    for t in range(gn):
                        nc.tensor.transpose(
                            ptg[:, 128 * t:128 * t + TB],
                            xab[:, (g0 + t) * U:(g0 + t + 1) * U],
                            ident[:TB, :TB])
                    src = ptg[:].rearrange("p (g v) -> p g v", v=128)[:, 0:gn, 0:TB]
                    dst = xt[:, PAD + g0 * TB:PAD + (g0 + gn) * TB].rearrange(
                        "p (g v) -> p g v", v=TB)
                    nc.vector.tensor_copy(dst, src)
                return xt

            def emit_fir(ch, xt):
                # ---- FIR: X-stationary paired-block matmuls
                yn = yn_p.tile([TB, QB * U // TB], BF16)  # [125, 1280]
                last = ch == CH - 1
                # last channel: spread the PSUM->yn copies over three engines
                # and split the out-DMA, shortening the serial tail after the
                # final matmul (everything else has drained by then)
                # (gpsimd cannot read PSUM, so the third engine is ACT again)
                copy_engines = (
                    (nc.scalar, nc.vector, nc.scalar) if last
                    else (nc.scalar, nc.scalar, nc.scalar))
                for gi, (g0, gcnt) in enumerate(((0, 2), (2, 2), (4, 1))):
                    py = fir_ps.tile([TB, 512], F32, tag="fir")
                    for j in range(gcnt):
                        b0 = (g0 + j) * GB
                        for h in (0, 1):
                            out_ap = py[:, j * 256 + h * U:j * 256 + (h + 1) * U]
                            for s in range(NTAP):
                                c0 = PAD + b0 + h - s
                                lhsT = xt[:, c0:c0 + 2 * TB].rearrange(
                                    "k (p two) -> k two p", two=2)[:, 0, :]
                                nc.tensor.matmul(
                                    out_ap, lhsT, cmat[:, s * U:(s + 1) * U],
                                    start=(s == 0), stop=(s == NTAP - 1))
                    eng = copy_engines[gi]
                    if eng is nc.scalar:
                        eng.copy(
                            yn[:, g0 * 256:(g0 + gcnt) * 256], py[:, :gcnt * 256])
                    else:
                        eng.tensor_copy(
                            yn[:, g0 * 256:(g0 + gcnt) * 256], py[:, :gcnt * 256])
                    if last:
                        nc.sync.dma_start(
                            y_r[ch][:, g0:g0 + gcnt],
                            yn[:, g0 * 256:(g0 + gcnt) * 256].rearrange(
                                "p (g i) -> p g i", i=2 * U))
                if not last:
                    nc.sync.dma_start(
                        y_r[ch], yn[:].rearrange("p (g i) -> p g i", i=2 * U))

            # software-pipeline the PE stream: channel ch+1's transposes are
            # emitted before channel ch's FIR so the PE never sits out the
            # DVE copy that completes ch's xt
            xts = {}
            for ch in range(CH):
                xts[ch] = emit_transpose(ch)
                if ch > 0:
                    emit_fir(ch - 1, xts.pop(ch - 1))
            emit_fir(CH - 1, xts.pop(CH - 1))

    nc.compile()
    return nc


_CACHE = {}


def _get_program():
    if "nc" not in _CACHE:
        _CACHE["nc"] = _build_program()
        import ml_dtypes
        _CACHE["cmat"] = _toeplitz_weights()
        _CACHE["ident"] = np.eye(U, dtype=ml_dtypes.bfloat16)
    return _CACHE["nc"], _CACHE["cmat"], _CACHE["ident"]


def kernel(waveform: np.ndarray, _trace: bool = False) -> np.ndarray:
    nc, cmat, ident = _get_program()
    x = np.ascontiguousarray(np.asarray(waveform), dtype=np.float32)
    assert x.shape == (C_TOTAL, T_TOTAL)
    shards = x.reshape(N_CORES, CH, T_TOTAL)
    in_maps = [{"x": shards[c], "cmat": cmat, "ident": ident} for c in range(N_CORES)]
    if _trace:
        try:
            res = run_bass_kernel_spmd(
                nc, in_maps, core_ids=list(range(N_CORES)), trace=True)
            kernel.last_exec_time_ns = res.exec_time_ns
            return np.concatenate(
                [np.asarray(r["y"]).astype(np.float32) for r in res.results], axis=0)
        except Exception:
            kernel.last_exec_time_ns = None
    res = run_bass_kernel_spmd(nc, in_maps, core_ids=list(range(N_CORES)))
    return np.concatenate(
        [np.asarray(r["y"]).astype(np.float32) for r in res.results], axis=0)


# revision 17
# speedup vs baseline: 2.8254x; 1.0015x over previous
"""Trainium2 Bass kernel for nn_MicResponseAugment: HP(125Hz)+LP(6kHz) biquad
cascade over waveform [128, 160000] f32.

Algorithm: the biquad cascade is an LTI filter; its impulse response decays
like r^n (r = 0.9659), so a truncated causal FIR computed as block-Toeplitz
matmuls on the PE replaces the sequential IIR scan.  All FIR arithmetic is
bf16 (inputs, taps, outputs) with f32 PSUM accumulation: measured rel err
5.1e-3 against the f32 reference, dominated by bf16 quantization — well
under the 2e-2 gate — and 4x-8x cheaper on every engine than the fp32/f32r
mix.

Dataflow per channel (16 channels/core, data-parallel over 8 cores):
  1. one 640KB DMA in: xa[125 p, 10*128] f32 (block q = t*125+p of 128
     samples each; 512B-contiguous descriptors -> full 360 GB/s)
  2. 10 PE transposes (f32, 2 cyc/row) -> PSUM, batched 4/4/2 per bank
  3. DVE copy PSUM -> xt bf16 [128 k, 2+1250 q] (cast in the copy)
  4. FIR as X-stationary matmuls: stationary = stride-2 column windows of
     xt (125 block-pairs), moving = Toeplitz tap blocks C_s bf16 [128,128],
     s=0,1 (taps 0..255; coverage >= 129 taps/sample, truncation noise
     ~1e-2 of bf16 noise).  Output PSUM tile [125, 512] holds TWO groups of
     250 blocks: partition p carries 256 *consecutive* samples
  5. ACT copy PSUM -> yn bf16 [125, 1280] (cast)
  6. one 320KB DMA out (512B-contiguous bf16 runs -> full bandwidth);
     host upcasts bf16 -> f32

All 16 input DMAs are issued up front: the DMA engines are the roofline
(10.24MB in + 5.12MB out = 42.7us at 360 GB/s), and front-loading keeps the
PE fed back-to-back so it stays at its ramped clock (matmul cost doubles if
the pipeline restarts each channel).
"""

import numpy as np
from contextlib import ExitStack

import concourse.bacc as bacc
import concourse.bass as bass
import concourse.tile as tile
from concourse import mybir
from concourse.bass_utils import run_bass_kernel_spmd

# ---------------------------------------------------------------- constants
SR = 16000
HP_FREQ = 125.0
LP_FREQ = 6000.0
Q_FACT = 0.7071067811865476

N_CORES = 8
C_TOTAL = 128
T_TOTAL = 160000
CH = C_TOTAL // N_CORES          # 16 channels per core
U = 128                          # FIR block length
QB = T_TOTAL // U                # 1250 blocks per channel
TB = 125                         # blocks per transpose tile
NT = QB // TB                    # 10 transpose tiles per channel
PAD = 2                          # zero-history columns per channel
NTAP = 2                         # tap blocks: taps 0..255
NG = 5                           # output groups of 250 blocks (2 per psum bank)
GB = QB // NG                    # 250 blocks per output group
TGROUPS = [(0, 4), (4, 4), (8, 2)]

F32 = mybir.dt.float32
BF16 = mybir.dt.bfloat16


def _impulse_response(n: int) -> np.ndarray:
    """Cascade impulse response, float64 (from fp32-rounded coefficients)."""
    def coeffs(freq, highpass):
        w0 = 2.0 * np.pi * freq / SR
        cw, sw = np.cos(w0), np.sin(w0)
        al = sw / (2.0 * Q_FACT)
        if highpass:
            b = np.array([(1 + cw) / 2, -(1 + cw), (1 + cw) / 2])
        else:
            b = np.array([(1 - cw) / 2, (1 - cw), (1 - cw) / 2])
        a = np.array([1 + al, -2 * cw, 1 - al])
        b = (b / a[0]).astype(np.float32).astype(np.float64)
        a = (a / a[0]).astype(np.float32).astype(np.float64)
        return b, a

    def filt(x, b, a):
        y = np.zeros_like(x)
        for i in range(len(x)):
            acc = b[0] * x[i]
            if i >= 1:
                acc += b[1] * x[i - 1] - a[1] * y[i - 1]
            if i >= 2:
                acc += b[2] * x[i - 2] - a[2] * y[i - 2]
            y[i] = acc
        return y

    bh, ah = coeffs(HP_FREQ, True)
    bl, al = coeffs(LP_FREQ, False)
    x = np.zeros(n)
    x[0] = 1.0
    return filt(filt(x, bh, ah), bl, al)


def _toeplitz_weights() -> np.ndarray:
    """cmat[k, s*128 + i] = h[s*128 + i - k], shape [128, 256] bf16-valued f32.

    Stored in DRAM as uint16 (bf16 bit pattern) to avoid host-side ml_dtypes
    dependence in the DMA path; the dram tensor is declared bf16.
    """
    import ml_dtypes
    h = _impulse_response(NTAP * U)
    cmat = np.zeros((U, NTAP * U), dtype=np.float64)
    k = np.arange(U)[:, None]
    i = np.arange(U)[None, :]
    for s in range(NTAP):
        tau = s * U + i - k
        cmat[:, s * U:(s + 1) * U] = np.where(
            (tau >= 0) & (tau < NTAP * U), h[np.clip(tau, 0, NTAP * U - 1)], 0.0)
    return cmat.astype(np.float32).astype(ml_dtypes.bfloat16)


# ---------------------------------------------------------------- program
def _build_program():
    nc = bacc.Bacc("TRN2", target_bir_lowering=False, debug=False)
    x = nc.dram_tensor("x", [CH, T_TOTAL], F32, kind="ExternalInput")
    cmat_d = nc.dram_tensor("cmat", [U, NTAP * U], BF16, kind="ExternalInput")
    y = nc.dram_tensor("y", [CH, T_TOTAL], BF16, kind="ExternalOutput")

    # input view: block q = t*125 + p holds samples q*128 + u
    x_r = x.ap().rearrange("c (t p u) -> c p t u", t=NT, p=TB, u=U)
    # output view: partition p of group g holds samples (g*250+2p)*128 + i
    y_r = y.ap().rearrange("c (g p i) -> c p g i", g=NG, p=TB, i=2 * U)

    with tile.TileContext(nc) as tc:
        with ExitStack() as ctx:
            const_p = ctx.enter_context(tc.tile_pool(name="const", bufs=1))
            xa_p = ctx.enter_context(tc.tile_pool(name="xa", bufs=CH))
            xab_p = ctx.enter_context(tc.tile_pool(name="xab", bufs=3))
            xt_p = ctx.enter_context(tc.tile_pool(name="xt", bufs=3))
            # all yn bufs resident: out-DMAs queue behind the 16 front-loaded
            # input DMAs on the DMA engines, so compute must never block on a
            # yn buffer waiting for an out-DMA to retire it
            yn_p = ctx.enter_context(tc.tile_pool(name="yn", bufs=CH))
            ptg_ps = ctx.enter_context(tc.tile_pool(name="ptg", bufs=3, space="PSUM"))
            fir_ps = ctx.enter_context(tc.tile_pool(name="fir", bufs=3, space="PSUM"))

            # front-load every channel's input DMA (DMA engines are the
            # roofline; keeps PE continuously fed and at ramped clock).
            # Channel 0 goes first so the pipeline's head starts at the
            # earliest possible grant; the tiny const DMAs slot in behind it.
            # identity for PE transposes, built on the (idle) Pool engine so
            # it never touches the DMA critical path
            ident = const_p.tile([U, U], BF16)
            nc.gpsimd.memset(ident[:], 1.0)
            nc.gpsimd.affine_select(
                ident[:], ident[:], pattern=[[1, U]],
                compare_op=mybir.AluOpType.is_equal, fill=0.0,
                channel_multiplier=-1)
            cmat = const_p.tile([U, NTAP * U], BF16)
            xas = []
            for ch in range(CH):
                xa = xa_p.tile([TB, NT * U], F32)
                nc.sync.dma_start(
                    xa[:].rearrange("p (t u) -> p t u", u=U), x_r[ch])
                xas.append(xa)
                if ch == 0:
                    nc.sync.dma_start(cmat[:], cmat_d.ap()[:])

            def emit_transpose(ch):
                # ---- transpose to time-major: xt[k, PAD+q] bf16
                # +2 spare cols: the last stride-2 stationary window's slice
                # extends past q=1249 (only even offsets are addressed)
                # Pool (otherwise idle) pre-casts f32 -> bf16 so the PE
                # transposes run at 1 cyc/row instead of 2
                xa = xas[ch]
                xab = xab_p.tile([TB, NT * U], BF16)
                nc.gpsimd.tensor_copy(xab[:], xa[:])
                xt = xt_p.tile([U, PAD + QB + 2], BF16)
                nc.vector.memset(xt[:, 0:PAD], 0)
                for g0, gn in TGROUPS:
                    ptg = ptg_ps.tile([U, 512], BF16, tag="ptg", padded_shape=[U, 1024])
                    for t in range(gn):
                        nc.tensor.transpose(
                            ptg[:, 128 * t:128 * t + TB],
                            xab[:, (g0 + t) * U:(g0 + t + 1) * U],
                            ident[:TB, :TB])
                    src = ptg[:].rearrange("p (g v) -> p g v", v=128)[:, 0:gn, 0:TB]
                    dst = xt[:, PAD + g0 * TB:PAD + (g0 + gn) * TB].rearrange(
                        "p (g v) -> p g v", v=TB)
                    nc.vector.tensor_copy(dst, src)
                return xt

            def emit_fir(ch, xt):
                # ---- FIR: X-stationary paired-block matmuls
                yn = yn_p.tile([TB, QB * U // TB], BF16)  # [125, 1280]
                last = ch == CH - 1
                # last channel: spread the PSUM->yn copies over three engines
                # and split the out-DMA, shortening the serial tail after the
                # final matmul (everything else has drained by then)
                # (gpsimd cannot read PSUM, so the third engine is ACT again)
                copy_engines = (
                    (nc.scalar, nc.vector, nc.scalar) if last
                    else (nc.scalar, nc.scalar, nc.scalar))
                for gi, (g0, gcnt) in enumerate(((0, 2), (2, 2), (4, 1))):
                    py = fir_ps.tile([TB, 512], F32, tag="fir")
                    for j in range(gcnt):
                        b0 = (g0 + j) * GB
                        for h in (0, 1):
                            out_ap = py[:, j * 256 + h * U:j * 256 + (h + 1) * U]
                            for s in range(NTAP):
                                c0 = PAD + b0 + h - s
                                lhsT = xt[:, c0:c0 + 2 * TB].rearrange(
                                    "k (p two) -> k two p", two=2)[:, 0, :]
                                nc.tensor.matmul(
                                    out_ap, lhsT, cmat[:, s * U:(s + 1) * U],
                                    start=(s == 0), stop=(s == NTAP - 1))
                    eng = copy_engines[gi]
                    if eng is nc.scalar:
                        eng.copy(
                            yn[:, g0 * 256:(g0 + gcnt) * 256], py[:, :gcnt * 256])
                    else:
                        eng.tensor_copy(
                            yn[:, g0 * 256:(g0 + gcnt) * 256], py[:, :gcnt * 256])
                    if last:
                        nc.sync.dma_start(
                            y_r[ch][:, g0:g0 + gcnt],
                            yn[:, g0 * 256:(g0 + gcnt) * 256].rearrange(
                                "p (g i) -> p g i", i=2 * U))
                if not last:
                    nc.sync.dma_start(
                        y_r[ch], yn[:].rearrange("p (g i) -> p g i", i=2 * U))

            # software-pipeline the PE stream: channel ch+1's transposes are
            # emitted before channel ch's FIR so the PE never sits out the
            # DVE copy that completes ch's xt
            xts = {}
            for ch in range(CH):
                xts[ch] = emit_transpose(ch)
                if ch > 0:
                    emit_fir(ch - 1, xts.pop(ch - 1))
            emit_fir(CH - 1, xts.pop(CH - 1))

    nc.compile()
    return nc


_CACHE = {}


def _get_program():
    if "nc" not in _CACHE:
        _CACHE["nc"] = _build_program()
        _CACHE["cmat"] = _toeplitz_weights()
    return _CACHE["nc"], _CACHE["cmat"]


def kernel(waveform: np.ndarray, _trace: bool = False) -> np.ndarray:
    nc, cmat = _get_program()
    x = np.ascontiguousarray(np.asarray(waveform), dtype=np.float32)
    assert x.shape == (C_TOTAL, T_TOTAL)
    shards = x.reshape(N_CORES, CH, T_TOTAL)
    in_maps = [{"x": shards[c], "cmat": cmat} for c in range(N_CORES)]
    if _trace:
        try:
            res = run_bass_kernel_spmd(
                nc, in_maps, core_ids=list(range(N_CORES)), trace=True)
            kernel.last_exec_time_ns = res.exec_time_ns
            return np.concatenate(
                [np.asarray(r["y"]).astype(np.float32) for r in res.results], axis=0)
        except Exception:
            kernel.last_exec_time_ns = None
    res = run_bass_kernel_spmd(nc, in_maps, core_ids=list(range(N_CORES)))
    return np.concatenate(
        [np.asarray(r["y"]).astype(np.float32) for r in res.results], axis=0)
